# revision 5
# baseline (speedup 1.0000x reference)
"""Trainium2 Bass kernel for ConvLayer: 3x3 same-padding conv, N=32 C=192
H=W=56 Cout=384, fp32, + bias.

Strategy: data-parallel over batch across 8 NeuronCores (4 images/core, no
collectives). Per core the conv is an implicit GEMM on the TensorEngine:
for each of the 9 taps (kh, kw) and 2 contraction chunks of 128 input
channels, accumulate matmuls into PSUM. Output channels (384) are split
into 3 partition chunks of 128. The matmul free dim is a block of 8 output
rows x 56 cols = 448 pixels. Inputs are spatially zero-padded to 58x58 on
the host so tap slices are plain strided APs; C=192 is covered by chunks
[0:128] and [64:192] with the overlapping 64 channels zero-weighted in
chunk 1.
"""

import sys

sys.path.insert(0, "/opt/trn_rl_repo")

import numpy as np

import concourse.bass as bass  # noqa: F401  (bass must import before tile)
import concourse.tile as tile
from concourse import bacc, mybir
from concourse.bass_utils import run_bass_kernel_spmd

N, C, H, W_ = 32, 192, 56, 56
COUT = 384
NCORES = 8
NPC = N // NCORES  # images per core
HP, WP = H + 2, W_ + 2  # 58x58 padded
KC = 2  # contraction chunks (128 each, chunk1 = ch 64..192)
MC = 3  # cout chunks of 128
RB = 8  # output rows per psum block
NBLK = H // RB  # 7
NTAP = 9

# float32r: fp32 storage, single-pass full-rate PE mode (1 cycle/row at
# free dim >= 256).
MM_DT = mybir.dt.float32r

_NC_CACHE = {}


def _build():
    f32 = mybir.dt.float32
    nc = bacc.Bacc("TRN2", target_bir_lowering=False, debug=False)
    xin = nc.dram_tensor("x", [NPC, C, HP, WP], MM_DT, kind="ExternalInput").ap()
    win = nc.dram_tensor(
        "w", [128, KC, NTAP, MC, 128], MM_DT, kind="ExternalInput"
    ).ap()
    bin_ = nc.dram_tensor("b", [128, MC], f32, kind="ExternalInput").ap()
    out = nc.dram_tensor("out", [NPC, COUT, H, W_], f32, kind="ExternalOutput").ap()

    with tile.TileContext(nc) as tc:
        with (
            tc.tile_pool(name="wpool", bufs=1) as wpool,
            tc.tile_pool(name="xpool", bufs=2) as xpool,
            tc.tile_pool(name="opool", bufs=4) as opool,
            tc.tile_pool(name="ppool", bufs=8, space="PSUM") as ppool,
        ):
            w_t = wpool.tile([128, KC, NTAP, MC, 128], MM_DT)
            nc.sync.dma_start(w_t[:], win[:])
            b_t = wpool.tile([128, MC], f32)
            nc.sync.dma_start(b_t[:], bin_[:])

            for img in range(NPC):
                xb = xpool.tile([128, KC, HP, WP], MM_DT)
                nc.sync.dma_start(xb[:, 0], xin[img, 0:128])
                nc.sync.dma_start(xb[:, 1], xin[img, 64 : 64 + 128])
                for mc in range(MC):
                    psums = [
                        ppool.tile([128, RB, W_], f32, name=f"ps{i}", tag="ps")
                        for i in range(NBLK)
                    ]
                    t = 0
                    nmm = KC * NTAP
                    for kc in range(KC):
                        for kh in range(3):
                            for kw in range(3):
                                lhsT = w_t[:, kc, kh * 3 + kw, mc, :]
                                for blk in range(NBLK):
                                    rhs = xb[
                                        :,
                                        kc,
                                        blk * RB + kh : blk * RB + kh + RB,
                                        kw : kw + W_,
                                    ]
                                    nc.tensor.matmul(
                                        psums[blk][:],
                                        lhsT,
                                        rhs,
                                        start=(t == 0),
                                        stop=(t == nmm - 1),
                                    )
                                t += 1
                    for blk in range(NBLK):
                        ot = opool.tile([128, RB, W_], f32)
                        nc.scalar.activation(
                            ot[:],
                            psums[blk][:],
                            mybir.ActivationFunctionType.Identity,
                            bias=b_t[:, mc : mc + 1],
                        )
                        nc.sync.dma_start(
                            out[
                                img,
                                mc * 128 : (mc + 1) * 128,
                                blk * RB : (blk + 1) * RB,
                                :,
                            ],
                            ot[:],
                        )
    nc.compile()
    return nc


def _get_nc():
    if "nc" not in _NC_CACHE:
        _NC_CACHE["nc"] = _build()
    return _NC_CACHE["nc"]


def kernel(x, W, b):
    x = np.asarray(x, dtype=np.float32)
    W = np.asarray(W, dtype=np.float32)
    b = np.asarray(b, dtype=np.float32)

    nc = _get_nc()

    # Spatial zero-pad to 58x58.
    xp = np.zeros((N, C, HP, WP), np.float32)
    xp[:, :, 1 : H + 1, 1 : W_ + 1] = x

    # lhsT weight layout [ci_in_chunk, kc, tap, mc, co_in_chunk].
    # chunk0 = channels 0..128; chunk1 = channels 64..192 with the first 64
    # (already covered by chunk0) zero-weighted.
    wtr = W.transpose(1, 2, 3, 0).reshape(C, NTAP, MC, 128)
    wt = np.zeros((128, KC, NTAP, MC, 128), np.float32)
    wt[:, 0] = wtr[0:128]
    wt[64:128, 1] = wtr[128:192]
    wt = np.ascontiguousarray(wt)

    bh = np.ascontiguousarray(b.reshape(MC, 128).T)  # [co_in_chunk, mc]

    in_maps = [
        {"x": xp[i * NPC : (i + 1) * NPC], "w": wt, "b": bh} for i in range(NCORES)
    ]
    res = run_bass_kernel_spmd(nc, in_maps, core_ids=list(range(NCORES)))
    return np.concatenate(
        [res.results[i]["out"] for i in range(NCORES)], axis=0
    )


# revision 9
# speedup vs baseline: 1.1749x; 1.1749x over previous
"""Trainium2 Bass kernel for ConvLayer: 3x3 same-padding conv, N=32 C=192
H=W=56 Cout=384, fp32, + bias.

Strategy: data-parallel over batch across 8 NeuronCores (4 images/core, no
collectives). Per core the conv is an implicit GEMM on the TensorEngine.
The contraction folds (kh, ci) pairs into the partition dim: 3*192 = 576
values = 5 chunks of 128 (last chunk half zero-weighted), so each output
block needs 5 chunks x 3 kw taps = 15 accumulating matmuls instead of the
naive 9 taps x 2 channel chunks = 18. The kh row shift is baked into the
SBUF x layout at DMA time (chunk q, partition p holds rows shifted by the
pair's kh). Output channels (384) = 3 partition chunks of 128; matmul free
dim = 8 output rows x 56 cols = 448 pixels into one PSUM bank.

Inputs are spatially zero-padded to 58x58 on the host; weights are
pre-transposed on the host into the lhsT layout [pair_in_chunk, chunk, kw,
mc, cout_in_chunk]; both are typed float32r (fp32 bits, full-rate PE mode).
"""

import sys

sys.path.insert(0, "/opt/trn_rl_repo")

import numpy as np

import concourse.bass as bass  # noqa: F401
import concourse.tile as tile
from concourse import bacc, mybir
from concourse.bass_utils import run_bass_kernel_spmd

N, C, H, W_ = 32, 192, 56, 56
COUT = 384
NCORES = 8
NPC = N // NCORES  # images per core
HP, WP = H + 2, W_ + 2  # 58x58 padded
NQ = 5  # (kh, ci) contraction chunks of 128 (5*128 = 640 >= 576)
MC = 3  # cout chunks of 128
RB = 8  # output rows per psum block
NBLK = H // RB  # 7
NPAIR = 3 * C  # 576 (kh-major: pair = kh*C + ci)

MM_DT = mybir.dt.float32r

# Affine source pieces for each contraction chunk q: list of
# (dst_part_lo, dst_part_hi, kh, ci_lo, ci_hi). Chunk q covers pairs
# [128q, 128q+128); pair = kh*C + ci.
_CHUNK_PIECES = []
for _q in range(NQ):
    pieces = []
    lo, hi = _q * 128, _q * 128 + 128
    p = lo
    while p < hi:
        if p >= NPAIR:
            # zero-weighted tail: map to kh=2 data (weights are 0 there)
            pieces.append((p - lo, hi - lo, 2, C - (hi - p), C))
            break
        kh, ci = p // C, p % C
        run = min(hi, (kh + 1) * C) - p
        pieces.append((p - lo, p - lo + run, kh, ci, ci + run))
        p += run
    _CHUNK_PIECES.append(pieces)

_NC_CACHE = {}


def _build():
    f32 = mybir.dt.float32
    nc = bacc.Bacc("TRN2", target_bir_lowering=False, debug=False)
    xin = nc.dram_tensor("x", [NPC, C, HP, WP], MM_DT, kind="ExternalInput").ap()
    win = nc.dram_tensor("w", [128, MC, NQ, 3, 128], MM_DT, kind="ExternalInput").ap()
    bin_ = nc.dram_tensor("b", [128, MC], f32, kind="ExternalInput").ap()
    out = nc.dram_tensor("out", [NPC, COUT, H, W_], f32, kind="ExternalOutput").ap()

    with tile.TileContext(nc) as tc:
        with (
            tc.tile_pool(name="wpool", bufs=1) as wpool,
            tc.tile_pool(name="xpool", bufs=2) as xpool,
            tc.tile_pool(name="opool", bufs=4) as opool,
            tc.tile_pool(name="ppool", bufs=8, space="PSUM") as ppool,
        ):
            # Per-mc weight tiles so mc=0 matmuls start after a 1.2MB DMA.
            w_m = []
            for mc in range(MC):
                wt = wpool.tile([128, NQ, 3, 128], MM_DT, name=f"w{mc}", tag=f"w{mc}")
                nc.sync.dma_start(wt[:], win[:, mc])
                w_m.append(wt)
            b_t = wpool.tile([128, MC], f32)
            nc.sync.dma_start(b_t[:], bin_[:])

            for img in range(NPC):
                # Per-chunk x tiles: chunk q, partition p holds 56 rows of
                # xpad[ci] shifted down by the pair's kh, all 58 cols.
                xq = []
                for q in range(NQ):
                    xb = xpool.tile([128, H, WP], MM_DT, name=f"x{q}", tag=f"x{q}")
                    for (p0, p1, kh, c0, c1) in _CHUNK_PIECES[q]:
                        nc.sync.dma_start(
                            xb[p0:p1], xin[img, c0:c1, kh : kh + H, :]
                        )
                    xq.append(xb)
                for mc in range(MC):
                    psums = [
                        ppool.tile([128, RB, W_], f32, name=f"ps{i}", tag="ps")
                        for i in range(NBLK)
                    ]
                    t = 0
                    nmm = NQ * 3
                    for q in range(NQ):
                        for kw in range(3):
                            lhsT = w_m[mc][:, q, kw, :]
                            for blk in range(NBLK):
                                rhs = xq[q][
                                    :, blk * RB : blk * RB + RB, kw : kw + W_
                                ]
                                nc.tensor.matmul(
                                    psums[blk][:],
                                    lhsT,
                                    rhs,
                                    start=(t == 0),
                                    stop=(t == nmm - 1),
                                )
                            t += 1
                    for blk in range(NBLK):
                        ot = opool.tile([128, RB, W_], f32)
                        nc.scalar.activation(
                            ot[:],
                            psums[blk][:],
                            mybir.ActivationFunctionType.Identity,
                            bias=b_t[:, mc : mc + 1],
                        )
                        nc.sync.dma_start(
                            out[
                                img,
                                mc * 128 : (mc + 1) * 128,
                                blk * RB : (blk + 1) * RB,
                                :,
                            ],
                            ot[:],
                        )
    nc.compile()
    return nc


def _get_nc():
    if "nc" not in _NC_CACHE:
        _NC_CACHE["nc"] = _build()
    return _NC_CACHE["nc"]


def _prep_in_maps(x, W, b):
    x = np.asarray(x, dtype=np.float32)
    W = np.asarray(W, dtype=np.float32)
    b = np.asarray(b, dtype=np.float32)

    # Spatial zero-pad to 58x58.
    xp = np.zeros((N, C, HP, WP), np.float32)
    xp[:, :, 1 : H + 1, 1 : W_ + 1] = x

    # lhsT weights [pair_in_chunk, q, kw, mc, co]; pair = kh*C + ci.
    wtr = W.transpose(1, 2, 3, 0)  # [ci, kh, kw, co]
    wpairs = np.zeros((NQ * 128, 3, COUT), np.float32)  # [pair, kw, co]
    wpairs[:NPAIR] = wtr.transpose(1, 0, 2, 3).reshape(NPAIR, 3, COUT)
    wt = np.ascontiguousarray(
        wpairs.reshape(NQ, 128, 3, MC, 128).transpose(1, 3, 0, 2, 4)
    )

    bh = np.ascontiguousarray(b.reshape(MC, 128).T)  # [co_in_chunk, mc]

    return [
        {"x": xp[i * NPC : (i + 1) * NPC], "w": wt, "b": bh} for i in range(NCORES)
    ]


def kernel(x, W, b):
    nc = _get_nc()
    in_maps = _prep_in_maps(x, W, b)
    res = run_bass_kernel_spmd(nc, in_maps, core_ids=list(range(NCORES)))
    return np.concatenate(
        [res.results[i]["out"] for i in range(NCORES)], axis=0
    )
